# revision 8
# baseline (speedup 1.0000x reference)
"""Trainium2 Bass kernel for causal MHA with RoPE (nn_MHA_14164802142240).

Full-input contract: kernel(x, W_qkv, W_o) -> [B, S, E], distributed across
8 NeuronCores as (batch x head-group): core c handles batch c//4 and heads
(c%4)*4 .. (c%4)*4+3.  Each core computes its 4 heads' attention plus the
partial output projection over its W_o row block; the host sums the 4
head-group partials per batch (fp16 partials, fp32 sum).

v2 design (vs the 200us baseline):
- QKV phase is pipelined by seq-chunk: xT arrives via per-chunk 3D-pattern
  DMAs so matmuls start at ~2.5us and stream continuously (HAM stays warm);
  RoPE + per-head repack overlap the next chunk's matmuls.
- No duplicated Q/K rows: scores contract over 64 partitions (cycle count is
  free-dim-bound); repack volume halves.
- Causal-compacted scores: diagonal-straddling tiles write left-packed PSUM
  regions (no uninit strips, no memsets); PSUM is evacuated by vector/gpsimd
  copies into a per-(chunk,head) fp16 staging row, then ONE scalar exp
  instruction covers the whole row (16 exps total instead of 80) so the
  scalar engine stays under the PE roofline.
- Causal masking inside diagonal 128-blocks is a single constant [128,128]
  triangle multiply per block, after exp.
- Output written fp16 (halves tail DMA), proj interleaved into the head
  stream with 2-chunk lookahead so exp latency hides under scores matmuls.
"""

import numpy as np

B, S, E = 2, 2048, 1024
H, D = 16, 64
HG = 4          # heads per core
NCORES = 8
SC = 512        # q-chunk
NSC = S // SC   # 4
NST = S // 128  # 16 seq tiles
NE = E // 128   # 8 contraction chunks
VW = 66         # per-head V stationary width: 64 v cols + ones + pad
ESCALE = 0.125  # 1/sqrt(D)
EBIAS = -2.0    # exp(s*ESCALE + EBIAS); cancels in normalization

_COMPILED = None


def _build_bass():
    import concourse.bass as bass
    import concourse.mybir as mybir
    import concourse.tile as tile
    from concourse import bacc
    from contextlib import ExitStack

    f32 = mybir.dt.float32
    f16 = mybir.dt.float16
    Exp = mybir.ActivationFunctionType.Exp

    nc = bacc.Bacc("TRN2", target_bir_lowering=False, debug=False,
                   enable_asserts=False)

    xT_d = nc.dram_tensor("xT", [E, S], f16, kind="ExternalInput").ap()
    wqk_d = nc.dram_tensor("wqk", [E, 4 * 128], f16, kind="ExternalInput").ap()
    wv_d = nc.dram_tensor("wv", [E, HG * D], f16, kind="ExternalInput").ap()
    wo_d = nc.dram_tensor("wo", [HG * D, E], f16, kind="ExternalInput").ap()
    cs_d = nc.dram_tensor("cs", [128, 2 * S], f16, kind="ExternalInput").ap()
    tri_d = nc.dram_tensor("tri", [128, 128], f16, kind="ExternalInput").ap()
    out_d = nc.dram_tensor("out", [S, E], f16, kind="ExternalOutput").ap()

    with tile.TileContext(nc) as tc, ExitStack() as outer:
        pconst = outer.enter_context(tc.tile_pool(name="const", bufs=1))
        pv = outer.enter_context(tc.tile_pool(name="vbuf", bufs=1))
        pqk = outer.enter_context(tc.tile_pool(name="qkbuf", bufs=1))
        py = outer.enter_context(tc.tile_pool(name="ybuf", bufs=1))

        tri_t = pconst.tile([128, 128], f16, tag="tri")
        ebias_t = pconst.tile([128, 1], f32, tag="ebias")
        nc.gpsimd.memset(ebias_t[:], EBIAS)
        wo_t = [pconst.tile([128, E], f16, name=f"wo{ft}", tag=f"wo{ft}")
                for ft in range(2)]

        vt = [pv.tile([128, HG * VW], f16, name=f"v{st}", tag=f"v{st}")
              for st in range(NST)]
        # per-head q/k, 64 rows (x0 pairs 0:32, x1 pairs 32:64)
        qd = [pqk.tile([64, S], f16, name=f"qd{i}", tag=f"qd{i}")
              for i in range(HG)]
        kd = [pqk.tile([64, S], f16, name=f"kd{i}", tag=f"kd{i}")
              for i in range(HG)]
        # unnormalized y^T (fp32) and normalized fp16 version for the proj
        yT = [py.tile([128, S], f32, name=f"y{i}", tag=f"y{i}")
              for i in range(2)]
        yT2 = [py.tile([128, S], f16, name=f"y2{i}", tag=f"y2{i}")
               for i in range(2)]

        # ---------------- phase 1: QKV + RoPE + repack -------------------
        with ExitStack() as ph1:
            px = ph1.enter_context(tc.tile_pool(name="xt", bufs=1))
            pw = ph1.enter_context(tc.tile_pool(name="w", bufs=1))
            pqkraw = ph1.enter_context(tc.tile_pool(name="qkraw", bufs=1))
            ptmp = ph1.enter_context(tc.tile_pool(name="ropetmp", bufs=2))
            ps1 = ph1.enter_context(
                tc.tile_pool(name="ps1", bufs=1, space="PSUM"))

            xt = px.tile([128, NE * S], f16, tag="xt")
            wqk_t = pw.tile([128, NE * 512], f16, tag="wqk")
            wv_t = pw.tile([128, NE * HG * D], f16, tag="wv")
            cs_t = pw.tile([128, 2 * S], f16, tag="cs")
            # x0 cols 0:S, x1 cols S:2S
            qkraw_q = pqkraw.tile([128, 2 * S], f16, tag="qkq")
            qkraw_k = pqkraw.tile([128, 2 * S], f16, tag="qkk")

            xt_v = xt.rearrange("p (e s) -> p e s", e=NE)
            wqk_v = wqk_t.rearrange("p (e c) -> p e c", e=NE)
            wv_v = wv_t.rearrange("p (e c) -> p e c", e=NE)
            wqk_src = wqk_d.rearrange("(e p) c -> p e c", p=128)
            wv_src = wv_d.rearrange("(e p) c -> p e c", p=128)
            xT_src = xT_d.rearrange("(e p) s -> p e s", p=128)

            # input DMA issue order: gate the first matmuls on ~0.5MB
            nc.sync.dma_start(wqk_v[:, 0:4, :], wqk_src[:, 0:4, :])
            nc.gpsimd.dma_start(xt_v[:, 0:4, 0:SC], xT_src[:, 0:4, 0:SC])
            nc.sync.dma_start(wqk_v[:, 4:8, :], wqk_src[:, 4:8, :])
            nc.gpsimd.dma_start(xt_v[:, 4:8, 0:SC], xT_src[:, 4:8, 0:SC])
            nc.scalar.dma_start(wv_t[:], wv_src)
            nc.scalar.dma_start(cs_t[:], cs_d)
            for sc in range(1, NSC):
                q = nc.sync if sc % 2 == 1 else nc.gpsimd
                q.dma_start(xt_v[:, :, sc * SC:(sc + 1) * SC],
                            xT_src[:, :, sc * SC:(sc + 1) * SC])
            nc.sync.dma_start(tri_t[:], tri_d)
            for ft in range(2):
                nc.sync.dma_start(wo_t[ft][:], wo_d[ft * 128:(ft + 1) * 128, :])

            # ones/pad columns of v stationaries (during initial DMA wait)
            for st in range(NST):
                v_view = vt[st].rearrange("p (h w) -> p h w", h=HG)
                nc.gpsimd.memset(v_view[:, :, D:D + 1], 1.0)
                nc.gpsimd.memset(v_view[:, :, D + 1:VW], 0.0)

            cos = cs_t[:, 0:S]
            sin = cs_t[:, S:2 * S]

            for sc in range(NSC):
                ssl = slice(sc * SC, (sc + 1) * SC)
                # qk matmuls: pss[jt] = wqk[:,jt-block].T @ x  -> [128, SC]
                pss = [ps1.tile([128, SC], f32, name=f"pss{jt}",
                                tag=f"pss{jt}") for jt in range(4)]
                for e in range(NE):
                    for jt in range(4):
                        nc.tensor.matmul(
                            pss[jt][:],
                            lhsT=wqk_t[:, e * 512 + jt * 128:
                                       e * 512 + (jt + 1) * 128],
                            rhs=xt[:, e * S + sc * SC:e * S + (sc + 1) * SC],
                            start=(e == 0), stop=(e == NE - 1))
                # v matmuls: psv[j] = x[:,st-block].T @ wv -> [128, 256]
                psv = [ps1.tile([128, HG * D], f32, name=f"psv{j}",
                                tag=f"psv{j}") for j in range(4)]
                for e in range(NE):
                    for j in range(4):
                        st = 4 * sc + j
                        nc.tensor.matmul(
                            psv[j][:],
                            lhsT=xt[:, e * S + st * 128:e * S + st * 128 + 128],
                            rhs=wv_t[:, e * 256:(e + 1) * 256],
                            start=(e == 0), stop=(e == NE - 1))
                # evacuate qk psum into qkraw halves (gpsimd cannot
                # read PSUM; scalar is otherwise idle in phase 1)
                nc.vector.tensor_copy(qkraw_q[:, sc * SC:(sc + 1) * SC],
                                      pss[0][:])
                nc.scalar.copy(qkraw_q[:, S + sc * SC:S + (sc + 1) * SC],
                               pss[1][:])
                nc.vector.tensor_copy(qkraw_k[:, sc * SC:(sc + 1) * SC],
                                      pss[2][:])
                nc.scalar.copy(qkraw_k[:, S + sc * SC:S + (sc + 1) * SC],
                               pss[3][:])
                # evacuate v psum into vt stationaries
                for j in range(4):
                    st = 4 * sc + j
                    v_view = vt[st].rearrange("p (h w) -> p h w", h=HG)
                    if j % 2 == 0:
                        nc.vector.tensor_copy(
                            v_view[:, :, 0:D],
                            psv[j].rearrange("p (h d) -> p h d", h=HG))
                    else:
                        nc.scalar.copy(
                            v_view[:, :, 0:D],
                            psv[j].rearrange("p (h d) -> p h d", h=HG))
                # RoPE in place on this chunk's columns
                for t in (qkraw_q, qkraw_k):
                    x0 = t[:, sc * SC:(sc + 1) * SC]
                    x1 = t[:, S + sc * SC:S + (sc + 1) * SC]
                    c_ = cos[:, ssl]
                    s_ = sin[:, ssl]
                    tmp = ptmp.tile([128, SC], f16, tag="rt0")
                    tmp2 = ptmp.tile([128, SC], f16, tag="rt1")
                    nc.gpsimd.tensor_mul(tmp[:], x0, s_)     # x0*sin
                    nc.gpsimd.tensor_mul(tmp2[:], x1, s_)    # x1*sin
                    nc.gpsimd.tensor_mul(x0, x0, c_)         # x0*cos
                    nc.gpsimd.tensor_mul(x1, x1, c_)         # x1*cos
                    nc.gpsimd.tensor_sub(x0, x0, tmp2[:])    # x0 c - x1 s
                    nc.gpsimd.tensor_add(x1, x1, tmp[:])     # x0 s + x1 c
                # repack after each sc-pair: qd/kd[h] rows [x0_h; x1_h]
                if sc % 2 == 1:
                    pc = slice((sc - 1) * SC, (sc + 1) * SC)
                    # scalar only in the first pair: it must be free for the
                    # first exp right after phase 1
                    qs = ([nc.sync, nc.scalar, nc.gpsimd, nc.sync] if sc == 1
                          else [nc.sync, nc.gpsimd, nc.sync, nc.gpsimd])
                    for h in range(HG):
                        sl = slice(32 * h, 32 * h + 32)
                        qs[h % 4].dma_start(qd[h][0:32, pc], qkraw_q[sl, pc])
                        qs[(h + 1) % 4].dma_start(
                            qd[h][32:64, pc],
                            qkraw_q[sl, S + (sc - 1) * SC:S + (sc + 1) * SC])
                        qs[(h + 2) % 4].dma_start(kd[h][0:32, pc],
                                                  qkraw_k[sl, pc])
                        qs[(h + 3) % 4].dma_start(
                            kd[h][32:64, pc],
                            qkraw_k[sl, S + (sc - 1) * SC:S + (sc + 1) * SC])

        # ---------------- phase 2: attention + projection ----------------
        # head stream: i = 4*c + h; PE order uses 2-slot lookahead so the
        # per-(c,h) exp hides under the next chunks' scores matmuls.
        with ExitStack() as ph2:
            psraw = ph2.enter_context(tc.tile_pool(name="sraw", bufs=3))
            psm = ph2.enter_context(tc.tile_pool(name="small", bufs=3))
            pob = ph2.enter_context(tc.tile_pool(name="outbuf", bufs=4))
            ps_s = ph2.enter_context(
                tc.tile_pool(name="ps_s", bufs=2, space="PSUM"))
            ps_y = ph2.enter_context(
                tc.tile_pool(name="ps_y", bufs=2, space="PSUM"))
            ps_o = ph2.enter_context(
                tc.tile_pool(name="ps_o", bufs=2, space="PSUM"))

            sraws = {}
            evac_i = 0
            ob_i = 0

            def block_off(c, t):
                """sraw column offset + width for k-tile t of chunk c."""
                g = t - 4 * c
                if g < 0:
                    return 512 * t, 512, 0
                offs = (0, 512, 896, 1152)
                return 2048 * c + offs[g], 512 - 128 * g, 128 * g

            def emit_scores(i):
                nonlocal evac_i
                c, h = i // HG, i % HG
                W = 2048 * c + 1280
                sraw = psraw.tile([128, 7424], f16, tag="sraw")
                sraws[i] = sraw
                npair = 2 * c + 2
                for pi in range(npair):
                    pss2 = ps_s.tile([128, 2 * SC], f32, tag="pss2")
                    po = []
                    for half in range(2):
                        t = 2 * pi + half
                        off, n, rg = block_off(c, t)
                        dst_off = 0 if half == 0 else po[0][1]
                        nc.tensor.matmul(
                            pss2[:, dst_off:dst_off + n],
                            lhsT=kd[h][:, t * 128:(t + 1) * 128],
                            rhs=qd[h][:, c * SC + rg:(c + 1) * SC],
                            start=True, stop=True)
                        po.append((off, n))
                    # evacuate pair -> sraw (compact): engines 5:3 vec:gp
                    wtot = po[0][1] + po[1][1]
                    evac_i += 1
                    nc.vector.tensor_copy(sraw[:, po[0][0]:po[0][0] + wtot],
                                          pss2[:, 0:wtot])
                # one exp over the whole compact row
                nc.scalar.activation(sraw[:, 0:W], sraw[:, 0:W], Exp,
                                     bias=ebias_t[:], scale=ESCALE)
                # triangle masks on the 4 diagonal blocks
                for g in range(4):
                    off, n, rg = block_off(c, 4 * c + g)
                    nc.gpsimd.tensor_mul(sraw[:, off:off + 128],
                                         sraw[:, off:off + 128], tri_t[:])

            def emit_y(i):
                c, h = i // HG, i % HG
                sraw = sraws.pop(i)
                nt = 4 * c + 4
                psy = ps_y.tile([128, SC], f32, tag="psy")
                for t in range(nt):
                    off, n, rg = block_off(c, t)
                    nc.tensor.matmul(
                        psy[0:VW, rg:SC],
                        lhsT=vt[t][:, VW * h:VW * (h + 1)],
                        rhs=sraw[:, off:off + n],
                        start=(t == 0), stop=(t == nt - 1))
                # normalization: denom row -> recip -> broadcast -> scale
                ro = 64 * (h % 2)
                nc.vector.tensor_copy(
                    yT[h // 2][ro:ro + 64, c * SC:(c + 1) * SC], psy[0:D, :])
                lrow = psm.tile([1, SC], f32, tag="lrow")
                nc.vector.tensor_copy(lrow[:], psy[D:D + 1, :])
                rrow = psm.tile([1, SC], f32, tag="rrow")
                nc.vector.reciprocal_approx_fast(rrow[:], lrow[:])
                rbc = psm.tile([128, SC], f32, tag="rbc")
                nc.gpsimd.partition_broadcast(rbc[:], rrow[:])
                nc.gpsimd.tensor_mul(
                    yT2[h // 2][ro:ro + 64, c * SC:(c + 1) * SC],
                    yT[h // 2][ro:ro + 64, c * SC:(c + 1) * SC],
                    rbc[ro:ro + 64, :])

            def emit_proj(c):
                nonlocal ob_i
                for st in range(4 * c, 4 * c + 4):
                    for ec in range(2):
                        pso = ps_o.tile([128, SC], f32, tag="pso")
                        for ft in range(2):
                            nc.tensor.matmul(
                                pso[:],
                                lhsT=yT2[ft][:, st * 128:(st + 1) * 128],
                                rhs=wo_t[ft][:, ec * SC:(ec + 1) * SC],
                                start=(ft == 0), stop=(ft == 1))
                        ob = pob.tile([128, SC], f16, tag="ob")
                        nc.vector.tensor_copy(ob[:], pso[:])
                        (nc.sync if ob_i % 2 == 0 else nc.gpsimd).dma_start(
                            out_d[st * 128:(st + 1) * 128,
                                  ec * SC:(ec + 1) * SC],
                            ob[:])
                        ob_i += 1

            # stream: s(0), s(1), then for i: s(i+2), y(i), proj between
            emit_scores(0)
            emit_scores(1)
            for i in range(16):
                if i + 2 < 16:
                    emit_scores(i + 2)
                emit_y(i)
                if i % HG == HG - 1:
                    emit_proj(i // HG)

    nc.compile()
    return nc


def _host_inputs(x, W_qkv, W_o):
    """Build the 8 per-core input maps (fp16 device-side compute dtypes)."""
    thetas = 10000.0 ** (-2.0 * (np.arange(D // 2, dtype=np.float32) / D))
    freqs = np.arange(S, dtype=np.float32)[:, None] * thetas[None, :]  # [S, 32]
    cosT = np.cos(freqs).astype(np.float32).T  # [32, S]
    sinT = np.sin(freqs).astype(np.float32).T
    cs = np.ascontiguousarray(np.concatenate(
        [np.tile(cosT, (4, 1)), np.tile(sinT, (4, 1))], axis=1)
        .astype(np.float16))  # [128, 2S]

    jj = np.arange(128)[:, None]
    tri = np.ascontiguousarray(
        (jj <= np.arange(128)[None, :]).astype(np.float16))  # [128, 128]

    xTs = [np.ascontiguousarray(x[b].T.astype(np.float16)) for b in range(B)]

    in_maps = []
    for core in range(NCORES):
        b, hg = core // 4, core % 4
        heads = range(hg * HG, (hg + 1) * HG)
        qx0 = [h * D + 2 * m for h in heads for m in range(D // 2)]
        qx1 = [h * D + 2 * m + 1 for h in heads for m in range(D // 2)]
        rows = (qx0 + qx1 + [E + i for i in qx0] + [E + i for i in qx1])
        wqk = np.ascontiguousarray(W_qkv[rows].T.astype(np.float16))  # [E, 512]
        vrows = [2 * E + h * D + d for h in heads for d in range(D)]
        wv = np.ascontiguousarray(W_qkv[vrows].T.astype(np.float16))  # [E, 256]
        wo = np.ascontiguousarray(
            W_o[:, hg * HG * D:(hg + 1) * HG * D].T.astype(np.float16))
        in_maps.append({
            "xT": xTs[b], "wqk": wqk, "wv": wv, "wo": wo,
            "cs": cs, "tri": tri,
        })
    return in_maps


def kernel(x, W_qkv, W_o):
    global _COMPILED
    x = np.ascontiguousarray(np.asarray(x, dtype=np.float32))
    W_qkv = np.ascontiguousarray(np.asarray(W_qkv, dtype=np.float32))
    W_o = np.ascontiguousarray(np.asarray(W_o, dtype=np.float32))

    if _COMPILED is None:
        _COMPILED = _build_bass()
    nc = _COMPILED

    from concourse.bass_utils import run_bass_kernel_spmd
    in_maps = _host_inputs(x, W_qkv, W_o)
    res = run_bass_kernel_spmd(nc, in_maps, core_ids=list(range(NCORES)))
    out = np.zeros((B, S, E), dtype=np.float32)
    for core in range(NCORES):
        out[core // 4] += res.results[core]["out"].astype(np.float32)
    return out


# revision 10
# speedup vs baseline: 1.8823x; 1.8823x over previous
"""Trainium2 Bass kernel for causal MHA with RoPE (nn_MHA_14164802142240).

Full-input contract: kernel(x, W_qkv, W_o) -> [B, S, E], distributed across
8 NeuronCores as (batch x head-group): core c handles batch c//4 and heads
(c%4)*4 .. (c%4)*4+3.  Each core computes its 4 heads' attention plus the
partial output projection over its W_o row block; the host sums the 4
head-group partials per batch (fp16 partials, fp32 sum).

v2 design (vs the 200us baseline):
- QKV phase is pipelined by seq-chunk: xT arrives via per-chunk 3D-pattern
  DMAs so matmuls start at ~2.5us and stream continuously (HAM stays warm);
  RoPE + per-head repack overlap the next chunk's matmuls.
- No duplicated Q/K rows: scores contract over 64 partitions (cycle count is
  free-dim-bound); repack volume halves.
- Causal-compacted scores: diagonal-straddling tiles write left-packed PSUM
  regions (no uninit strips, no memsets); PSUM is evacuated by vector/gpsimd
  copies into a per-(chunk,head) fp16 staging row, then ONE scalar exp
  instruction covers the whole row (16 exps total instead of 80) so the
  scalar engine stays under the PE roofline.
- Causal masking inside diagonal 128-blocks is a single constant [128,128]
  triangle multiply per block, after exp.
- Output written fp16 (halves tail DMA), proj interleaved into the head
  stream with 2-chunk lookahead so exp latency hides under scores matmuls.
"""

import numpy as np

B, S, E = 2, 2048, 1024
H, D = 16, 64
HG = 4          # heads per core
NCORES = 8
SC = 512        # q-chunk
NSC = S // SC   # 4
NST = S // 128  # 16 seq tiles
NE = E // 128   # 8 contraction chunks
VW = 66         # per-head V stationary width: 64 v cols + ones + pad
ESCALE = 0.125  # 1/sqrt(D)
EBIAS = -2.0    # exp(s*ESCALE + EBIAS); cancels in normalization

_COMPILED = None


def _build_bass():
    import concourse.bass as bass
    import concourse.mybir as mybir
    import concourse.tile as tile
    from concourse import bacc
    from contextlib import ExitStack

    f32 = mybir.dt.float32
    f16 = mybir.dt.float16
    Exp = mybir.ActivationFunctionType.Exp

    nc = bacc.Bacc("TRN2", target_bir_lowering=False, debug=False,
                   enable_asserts=False)

    xT_d = nc.dram_tensor("xT", [E, S], f16, kind="ExternalInput").ap()
    wqk_d = nc.dram_tensor("wqk", [E, 4 * 128], f16, kind="ExternalInput").ap()
    wv_d = nc.dram_tensor("wv", [E, HG * D], f16, kind="ExternalInput").ap()
    wo_d = nc.dram_tensor("wo", [HG * D, E], f16, kind="ExternalInput").ap()
    cs_d = nc.dram_tensor("cs", [128, 2 * S], f16, kind="ExternalInput").ap()
    tri_d = nc.dram_tensor("tri", [128, 128], f16, kind="ExternalInput").ap()
    out_d = nc.dram_tensor("out", [S, E], f16, kind="ExternalOutput").ap()

    with tile.TileContext(nc) as tc, ExitStack() as outer:
        pconst = outer.enter_context(tc.tile_pool(name="const", bufs=1))
        pv = outer.enter_context(tc.tile_pool(name="vbuf", bufs=1))
        pqk = outer.enter_context(tc.tile_pool(name="qkbuf", bufs=1))
        py = outer.enter_context(tc.tile_pool(name="ybuf", bufs=1))

        tri_t = pconst.tile([128, 128], f16, tag="tri")
        ebias_t = pconst.tile([128, 1], f32, tag="ebias")
        nc.gpsimd.memset(ebias_t[:], EBIAS)
        wo_t = [pconst.tile([128, E], f16, name=f"wo{ft}", tag=f"wo{ft}")
                for ft in range(2)]

        vt = [pv.tile([128, HG * VW], f16, name=f"v{st}", tag=f"v{st}")
              for st in range(NST)]
        # per-head q/k, 64 rows (x0 pairs 0:32, x1 pairs 32:64)
        qd = [pqk.tile([64, S], f16, name=f"qd{i}", tag=f"qd{i}")
              for i in range(HG)]
        kd = [pqk.tile([64, S], f16, name=f"kd{i}", tag=f"kd{i}")
              for i in range(HG)]
        # unnormalized y^T (fp32) and normalized fp16 version for the proj
        yT = [py.tile([128, S], f32, name=f"y{i}", tag=f"y{i}")
              for i in range(2)]
        yT2 = [py.tile([128, S], f16, name=f"y2{i}", tag=f"y2{i}")
               for i in range(2)]

        # ---------------- phase 1: QKV + RoPE + repack -------------------
        with ExitStack() as ph1:
            px = ph1.enter_context(tc.tile_pool(name="xt", bufs=1))
            pw = ph1.enter_context(tc.tile_pool(name="w", bufs=1))
            pqkraw = ph1.enter_context(tc.tile_pool(name="qkraw", bufs=1))
            ptmp = ph1.enter_context(tc.tile_pool(name="ropetmp", bufs=2))
            ps1 = ph1.enter_context(
                tc.tile_pool(name="ps1", bufs=1, space="PSUM"))

            xt = px.tile([128, NE * S], f16, tag="xt")
            wqk_t = pw.tile([128, NE * 512], f16, tag="wqk")
            wv_t = pw.tile([128, NE * HG * D], f16, tag="wv")
            cs_t = pw.tile([128, 2 * S], f16, tag="cs")
            # x0 cols 0:S, x1 cols S:2S
            qkraw_q = pqkraw.tile([128, 2 * S], f16, tag="qkq")
            qkraw_k = pqkraw.tile([128, 2 * S], f16, tag="qkk")

            xt_v = xt.rearrange("p (e s) -> p e s", e=NE)
            wqk_v = wqk_t.rearrange("p (e c) -> p e c", e=NE)
            wv_v = wv_t.rearrange("p (e c) -> p e c", e=NE)
            wqk_src = wqk_d.rearrange("(e p) c -> p e c", p=128)
            wv_src = wv_d.rearrange("(e p) c -> p e c", p=128)
            xT_src = xT_d.rearrange("(e p) s -> p e s", p=128)

            # input DMA issue order: gate the first matmuls on ~0.5MB
            nc.sync.dma_start(wqk_v[:, 0:4, :], wqk_src[:, 0:4, :])
            nc.gpsimd.dma_start(xt_v[:, 0:4, 0:SC], xT_src[:, 0:4, 0:SC])
            nc.sync.dma_start(wqk_v[:, 4:8, :], wqk_src[:, 4:8, :])
            nc.gpsimd.dma_start(xt_v[:, 4:8, 0:SC], xT_src[:, 4:8, 0:SC])
            nc.scalar.dma_start(wv_t[:], wv_src)
            nc.scalar.dma_start(cs_t[:], cs_d)
            for sc in range(1, NSC):
                q = nc.sync if sc % 2 == 1 else nc.gpsimd
                q.dma_start(xt_v[:, :, sc * SC:(sc + 1) * SC],
                            xT_src[:, :, sc * SC:(sc + 1) * SC])
            nc.sync.dma_start(tri_t[:], tri_d)
            for ft in range(2):
                nc.sync.dma_start(wo_t[ft][:], wo_d[ft * 128:(ft + 1) * 128, :])

            # ones/pad columns of v stationaries (during initial DMA wait)
            for st in range(NST):
                v_view = vt[st].rearrange("p (h w) -> p h w", h=HG)
                nc.gpsimd.memset(v_view[:, :, D:D + 1], 1.0)
                nc.gpsimd.memset(v_view[:, :, D + 1:VW], 0.0)

            cos = cs_t[:, 0:S]
            sin = cs_t[:, S:2 * S]

            for sc in range(NSC):
                ssl = slice(sc * SC, (sc + 1) * SC)
                # qk matmuls: pss[jt] = wqk[:,jt-block].T @ x  -> [128, SC]
                pss = [ps1.tile([128, SC], f32, name=f"pss{jt}",
                                tag=f"pss{jt}") for jt in range(4)]
                for e in range(NE):
                    for jt in range(4):
                        nc.tensor.matmul(
                            pss[jt][:],
                            lhsT=wqk_t[:, e * 512 + jt * 128:
                                       e * 512 + (jt + 1) * 128],
                            rhs=xt[:, e * S + sc * SC:e * S + (sc + 1) * SC],
                            start=(e == 0), stop=(e == NE - 1))
                # v matmuls: psv[j] = x[:,st-block].T @ wv -> [128, 256]
                psv = [ps1.tile([128, HG * D], f32, name=f"psv{j}",
                                tag=f"psv{j}") for j in range(4)]
                for e in range(NE):
                    for j in range(4):
                        st = 4 * sc + j
                        nc.tensor.matmul(
                            psv[j][:],
                            lhsT=xt[:, e * S + st * 128:e * S + st * 128 + 128],
                            rhs=wv_t[:, e * 256:(e + 1) * 256],
                            start=(e == 0), stop=(e == NE - 1))
                # evacuate qk psum into qkraw halves (gpsimd cannot
                # read PSUM; scalar is otherwise idle in phase 1)
                nc.vector.tensor_copy(qkraw_q[:, sc * SC:(sc + 1) * SC],
                                      pss[0][:])
                nc.scalar.copy(qkraw_q[:, S + sc * SC:S + (sc + 1) * SC],
                               pss[1][:])
                nc.vector.tensor_copy(qkraw_k[:, sc * SC:(sc + 1) * SC],
                                      pss[2][:])
                nc.scalar.copy(qkraw_k[:, S + sc * SC:S + (sc + 1) * SC],
                               pss[3][:])
                # evacuate v psum into vt stationaries
                for j in range(4):
                    st = 4 * sc + j
                    v_view = vt[st].rearrange("p (h w) -> p h w", h=HG)
                    if j % 2 == 0:
                        nc.vector.tensor_copy(
                            v_view[:, :, 0:D],
                            psv[j].rearrange("p (h d) -> p h d", h=HG))
                    else:
                        nc.scalar.copy(
                            v_view[:, :, 0:D],
                            psv[j].rearrange("p (h d) -> p h d", h=HG))
                # RoPE in place on this chunk's columns
                for t in (qkraw_q, qkraw_k):
                    x0 = t[:, sc * SC:(sc + 1) * SC]
                    x1 = t[:, S + sc * SC:S + (sc + 1) * SC]
                    c_ = cos[:, ssl]
                    s_ = sin[:, ssl]
                    tmp = ptmp.tile([128, SC], f16, tag="rt0")
                    tmp2 = ptmp.tile([128, SC], f16, tag="rt1")
                    nc.gpsimd.tensor_mul(tmp[:], x0, s_)     # x0*sin
                    nc.gpsimd.tensor_mul(tmp2[:], x1, s_)    # x1*sin
                    nc.gpsimd.tensor_mul(x0, x0, c_)         # x0*cos
                    nc.gpsimd.tensor_mul(x1, x1, c_)         # x1*cos
                    nc.gpsimd.tensor_sub(x0, x0, tmp2[:])    # x0 c - x1 s
                    nc.gpsimd.tensor_add(x1, x1, tmp[:])     # x0 s + x1 c
                # repack after each sc-pair: qd/kd[h] rows [x0_h; x1_h]
                if sc % 2 == 1:
                    pc = slice((sc - 1) * SC, (sc + 1) * SC)
                    # scalar only in the first pair: it must be free for the
                    # first exp right after phase 1
                    qs = ([nc.sync, nc.scalar, nc.gpsimd, nc.sync] if sc == 1
                          else [nc.sync, nc.gpsimd, nc.sync, nc.gpsimd])
                    for h in range(HG):
                        sl = slice(32 * h, 32 * h + 32)
                        qs[h % 4].dma_start(qd[h][0:32, pc], qkraw_q[sl, pc])
                        qs[(h + 1) % 4].dma_start(
                            qd[h][32:64, pc],
                            qkraw_q[sl, S + (sc - 1) * SC:S + (sc + 1) * SC])
                        qs[(h + 2) % 4].dma_start(kd[h][0:32, pc],
                                                  qkraw_k[sl, pc])
                        qs[(h + 3) % 4].dma_start(
                            kd[h][32:64, pc],
                            qkraw_k[sl, S + (sc - 1) * SC:S + (sc + 1) * SC])

        # ---------------- phase 2: attention + projection ----------------
        # per-pair pipeline (LOOKP decoupling): scores pair -> exp direct
        # from PSUM into fp16 pt (compact, no uninit strips) -> triangle
        # masks on diagonal blocks -> y matmuls 2 pairs behind.
        with ExitStack() as ph2:
            pp = ph2.enter_context(tc.tile_pool(name="pbuf", bufs=4))
            psm = ph2.enter_context(tc.tile_pool(name="small", bufs=3))
            pob = ph2.enter_context(tc.tile_pool(name="outbuf", bufs=4))
            ps_s = ph2.enter_context(
                tc.tile_pool(name="ps_s", bufs=2, space="PSUM"))
            ps_y = ph2.enter_context(
                tc.tile_pool(name="ps_y", bufs=1, space="PSUM"))
            ps_o = ph2.enter_context(
                tc.tile_pool(name="ps_o", bufs=2, space="PSUM"))

            LOOKP = 2
            ob_i = 0

            def pair_blocks(c, pi):
                """[(pt_off, n, rg, t), ...] for the two k-tiles of a pair."""
                out, dst = [], 0
                for half in range(2):
                    t = 2 * pi + half
                    rg = max(0, 128 * (t - 4 * c))
                    n = SC - rg
                    out.append((dst, n, rg, t))
                    dst += n
                return out

            def emit_proj(c):
                nonlocal ob_i
                for st in range(4 * c, 4 * c + 4):
                    for ec in range(2):
                        pso = ps_o.tile([128, SC], f32, tag="pso")
                        for ft in range(2):
                            nc.tensor.matmul(
                                pso[:],
                                lhsT=yT2[ft][:, st * 128:(st + 1) * 128],
                                rhs=wo_t[ft][:, ec * SC:(ec + 1) * SC],
                                start=(ft == 0), stop=(ft == 1))
                        ob = pob.tile([128, SC], f16, tag="ob")
                        nc.vector.tensor_copy(ob[:], pso[:])
                        (nc.sync if ob_i % 2 == 0 else nc.gpsimd).dma_start(
                            out_d[st * 128:(st + 1) * 128,
                                  ec * SC:(ec + 1) * SC],
                            ob[:])
                        ob_i += 1

            for i in range(16):
                c, h = i // HG, i % HG
                npair = 2 * c + 2
                nt = 4 * c + 4
                psy = ps_y.tile([128, SC], f32, tag="psy")
                pts = {}
                for pi in range(npair + LOOKP):
                    if pi < npair:
                        blocks = pair_blocks(c, pi)
                        wtot = blocks[-1][0] + blocks[-1][1]
                        pss2 = ps_s.tile([128, 2 * SC], f32, tag="pss2")
                        pt = pp.tile([128, 2 * SC], f16, tag="pt")
                        for (dst, n, rg, t) in blocks:
                            nc.tensor.matmul(
                                pss2[:, dst:dst + n],
                                lhsT=kd[h][:, t * 128:(t + 1) * 128],
                                rhs=qd[h][:, c * SC + rg:(c + 1) * SC],
                                start=True, stop=True)
                        nc.scalar.activation(pt[:, 0:wtot], pss2[:, 0:wtot],
                                             Exp, bias=ebias_t[:],
                                             scale=ESCALE)
                        for (dst, n, rg, t) in blocks:
                            if t >= 4 * c:  # diagonal block: triangle mask
                                nc.vector.tensor_mul(pt[:, dst:dst + 128],
                                                     pt[:, dst:dst + 128],
                                                     tri_t[:])
                        pts[pi] = (pt, blocks)
                    pp_ = pi - LOOKP
                    if 0 <= pp_ < npair:
                        pt, blocks = pts.pop(pp_)
                        for (dst, n, rg, t) in blocks:
                            nc.tensor.matmul(
                                psy[0:VW, rg:SC],
                                lhsT=vt[t][:, VW * h:VW * (h + 1)],
                                rhs=pt[:, dst:dst + n],
                                start=(t == 0), stop=(t == nt - 1))
                # normalization: denom -> recip -> broadcast -> scale
                ro = 64 * (h % 2)
                nc.vector.tensor_copy(
                    yT[h // 2][ro:ro + 64, c * SC:(c + 1) * SC], psy[0:D, :])
                lrow = psm.tile([1, SC], f32, tag="lrow")
                nc.vector.tensor_copy(lrow[:], psy[D:D + 1, :])
                rrow = psm.tile([1, SC], f32, tag="rrow")
                nc.vector.reciprocal_approx_fast(rrow[:], lrow[:])
                rbc = psm.tile([128, SC], f32, tag="rbc")
                nc.gpsimd.partition_broadcast(rbc[:], rrow[:])
                nc.vector.tensor_mul(
                    yT2[h // 2][ro:ro + 64, c * SC:(c + 1) * SC],
                    yT[h // 2][ro:ro + 64, c * SC:(c + 1) * SC],
                    rbc[ro:ro + 64, :])
                if h == HG - 1:
                    emit_proj(c)

    nc.compile()
    return nc


def _host_inputs(x, W_qkv, W_o):
    """Build the 8 per-core input maps (fp16 device-side compute dtypes)."""
    thetas = 10000.0 ** (-2.0 * (np.arange(D // 2, dtype=np.float32) / D))
    freqs = np.arange(S, dtype=np.float32)[:, None] * thetas[None, :]  # [S, 32]
    cosT = np.cos(freqs).astype(np.float32).T  # [32, S]
    sinT = np.sin(freqs).astype(np.float32).T
    cs = np.ascontiguousarray(np.concatenate(
        [np.tile(cosT, (4, 1)), np.tile(sinT, (4, 1))], axis=1)
        .astype(np.float16))  # [128, 2S]

    jj = np.arange(128)[:, None]
    tri = np.ascontiguousarray(
        (jj <= np.arange(128)[None, :]).astype(np.float16))  # [128, 128]

    xTs = [np.ascontiguousarray(x[b].T.astype(np.float16)) for b in range(B)]

    in_maps = []
    for core in range(NCORES):
        b, hg = core // 4, core % 4
        heads = range(hg * HG, (hg + 1) * HG)
        qx0 = [h * D + 2 * m for h in heads for m in range(D // 2)]
        qx1 = [h * D + 2 * m + 1 for h in heads for m in range(D // 2)]
        rows = (qx0 + qx1 + [E + i for i in qx0] + [E + i for i in qx1])
        wqk = np.ascontiguousarray(W_qkv[rows].T.astype(np.float16))  # [E, 512]
        vrows = [2 * E + h * D + d for h in heads for d in range(D)]
        wv = np.ascontiguousarray(W_qkv[vrows].T.astype(np.float16))  # [E, 256]
        wo = np.ascontiguousarray(
            W_o[:, hg * HG * D:(hg + 1) * HG * D].T.astype(np.float16))
        in_maps.append({
            "xT": xTs[b], "wqk": wqk, "wv": wv, "wo": wo,
            "cs": cs, "tri": tri,
        })
    return in_maps


def kernel(x, W_qkv, W_o):
    global _COMPILED
    x = np.ascontiguousarray(np.asarray(x, dtype=np.float32))
    W_qkv = np.ascontiguousarray(np.asarray(W_qkv, dtype=np.float32))
    W_o = np.ascontiguousarray(np.asarray(W_o, dtype=np.float32))

    if _COMPILED is None:
        _COMPILED = _build_bass()
    nc = _COMPILED

    from concourse.bass_utils import run_bass_kernel_spmd
    in_maps = _host_inputs(x, W_qkv, W_o)
    res = run_bass_kernel_spmd(nc, in_maps, core_ids=list(range(NCORES)))
    out = np.zeros((B, S, E), dtype=np.float32)
    for core in range(NCORES):
        out[core // 4] += res.results[core]["out"].astype(np.float32)
    return out


# revision 11
# speedup vs baseline: 2.0812x; 1.1057x over previous
"""Trainium2 Bass kernel for causal MHA with RoPE (nn_MHA_14164802142240).

Full-input contract: kernel(x, W_qkv, W_o) -> [B, S, E], distributed across
8 NeuronCores as (batch x head-group): core c handles batch c//4 and heads
(c%4)*4 .. (c%4)*4+3.  Each core computes its 4 heads' attention plus the
partial output projection over its W_o row block; the host sums the 4
head-group partials per batch (fp16 partials, fp32 sum).

v2 design (vs the 200us baseline):
- QKV phase is pipelined by seq-chunk: xT arrives via per-chunk 3D-pattern
  DMAs so matmuls start at ~2.5us and stream continuously (HAM stays warm);
  RoPE + per-head repack overlap the next chunk's matmuls.
- No duplicated Q/K rows: scores contract over 64 partitions (cycle count is
  free-dim-bound); repack volume halves.
- Causal-compacted scores: diagonal-straddling tiles write left-packed PSUM
  regions (no uninit strips, no memsets); PSUM is evacuated by vector/gpsimd
  copies into a per-(chunk,head) fp16 staging row, then ONE scalar exp
  instruction covers the whole row (16 exps total instead of 80) so the
  scalar engine stays under the PE roofline.
- Causal masking inside diagonal 128-blocks is a single constant [128,128]
  triangle multiply per block, after exp.
- Output written fp16 (halves tail DMA), proj interleaved into the head
  stream with 2-chunk lookahead so exp latency hides under scores matmuls.
"""

import numpy as np

B, S, E = 2, 2048, 1024
H, D = 16, 64
HG = 4          # heads per core
NCORES = 8
SC = 512        # q-chunk
NSC = S // SC   # 4
NST = S // 128  # 16 seq tiles
NE = E // 128   # 8 contraction chunks
VW = 66         # per-head V stationary width: 64 v cols + ones + pad
ESCALE = 0.125  # 1/sqrt(D)
EBIAS = -2.0    # exp(s*ESCALE + EBIAS); cancels in normalization

_COMPILED = None


def _build_bass():
    import concourse.bass as bass
    import concourse.mybir as mybir
    import concourse.tile as tile
    from concourse import bacc
    from contextlib import ExitStack

    f32 = mybir.dt.float32
    f16 = mybir.dt.float16
    Exp = mybir.ActivationFunctionType.Exp

    nc = bacc.Bacc("TRN2", target_bir_lowering=False, debug=False,
                   enable_asserts=False)

    # sc-major, (p,e)-row-ordered chunks: row sc*E + p*NE + e = x.T[e*128+p, sc*SC:]
    xT_d = nc.dram_tensor("xT", [NSC * E, SC], f16, kind="ExternalInput").ap()
    wqk_d = nc.dram_tensor("wqk", [E, 4 * 128], f16, kind="ExternalInput").ap()
    wv_d = nc.dram_tensor("wv", [E, HG * D], f16, kind="ExternalInput").ap()
    wo_d = nc.dram_tensor("wo", [HG * D, E], f16, kind="ExternalInput").ap()
    cs_d = nc.dram_tensor("cs", [128, 2 * S], f16, kind="ExternalInput").ap()
    tri_d = nc.dram_tensor("tri", [128, 128], f16, kind="ExternalInput").ap()
    out_d = nc.dram_tensor("out", [S, E], f16, kind="ExternalOutput").ap()

    with tile.TileContext(nc) as tc, ExitStack() as outer:
        pconst = outer.enter_context(tc.tile_pool(name="const", bufs=1))
        pv = outer.enter_context(tc.tile_pool(name="vbuf", bufs=1))
        pqk = outer.enter_context(tc.tile_pool(name="qkbuf", bufs=1))
        py = outer.enter_context(tc.tile_pool(name="ybuf", bufs=1))

        tri_t = pconst.tile([128, 128], f16, tag="tri")
        ebias_t = pconst.tile([128, 1], f32, tag="ebias")
        nc.gpsimd.memset(ebias_t[:], EBIAS)
        wo_t = [pconst.tile([128, E], f16, name=f"wo{ft}", tag=f"wo{ft}")
                for ft in range(2)]

        vt = [pv.tile([128, HG * VW], f16, name=f"v{st}", tag=f"v{st}")
              for st in range(NST)]
        # per-head q/k, 64 rows (x0 pairs 0:32, x1 pairs 32:64)
        qd = [pqk.tile([64, S], f16, name=f"qd{i}", tag=f"qd{i}")
              for i in range(HG)]
        kd = [pqk.tile([64, S], f16, name=f"kd{i}", tag=f"kd{i}")
              for i in range(HG)]
        # unnormalized y^T (fp32) and normalized fp16 version for the proj
        yT = [py.tile([128, S], f32, name=f"y{i}", tag=f"y{i}")
              for i in range(2)]
        yT2 = [py.tile([128, S], f16, name=f"y2{i}", tag=f"y2{i}")
               for i in range(2)]

        # ---------------- phase 1: QKV + RoPE + repack -------------------
        with ExitStack() as ph1:
            px = ph1.enter_context(tc.tile_pool(name="xt", bufs=1))
            pw = ph1.enter_context(tc.tile_pool(name="w", bufs=1))
            pqkraw = ph1.enter_context(tc.tile_pool(name="qkraw", bufs=1))
            ptmp = ph1.enter_context(tc.tile_pool(name="ropetmp", bufs=2))
            ps1 = ph1.enter_context(
                tc.tile_pool(name="ps1", bufs=1, space="PSUM"))

            xt = px.tile([128, NE * S], f16, tag="xt")
            wqk_t = pw.tile([128, NE * 512], f16, tag="wqk")
            wv_t = pw.tile([128, NE * HG * D], f16, tag="wv")
            cs_t = pw.tile([128, 2 * S], f16, tag="cs")
            # x0 cols 0:S, x1 cols S:2S
            qkraw_q = pqkraw.tile([128, 2 * S], f16, tag="qkq")
            qkraw_k = pqkraw.tile([128, 2 * S], f16, tag="qkk")

            xt_v = xt.rearrange("p (e s) -> p e s", e=NE)
            wqk_v = wqk_t.rearrange("p (e c) -> p e c", e=NE)
            wqk_src = wqk_d.rearrange("(e p) c -> p e c", p=128)
            wv_src = wv_d.rearrange("(e p) c -> p e c", p=128)
            # xT chunk sc: contiguous [E, SC] block, rows (p, e)-ordered, so
            # the DMA reads DRAM linearly into the strided SBUF view
            xT_src = xT_d.rearrange("(sc p e) s -> sc p e s", p=128, e=NE)
            xt_sc = [xt_v[:, :, sc * SC:(sc + 1) * SC].rearrange(
                "p e s -> p e s") for sc in range(NSC)]

            # input DMA issue order: gate the first matmuls on ~0.5MB
            nc.sync.dma_start(wqk_v[:, 0:4, :], wqk_src[:, 0:4, :])
            nc.gpsimd.dma_start(xt_v[:, 0:4, 0:SC], xT_src[0, :, 0:4, :])
            nc.sync.dma_start(wqk_v[:, 4:8, :], wqk_src[:, 4:8, :])
            nc.gpsimd.dma_start(xt_v[:, 4:8, 0:SC], xT_src[0, :, 4:8, :])
            nc.scalar.dma_start(wv_t[:], wv_src)
            nc.scalar.dma_start(cs_t[:], cs_d)
            for sc in range(1, NSC):
                q = nc.sync if sc % 2 == 1 else nc.gpsimd
                q.dma_start(xt_v[:, :, sc * SC:(sc + 1) * SC],
                            xT_src[sc, :, :, :])
            nc.sync.dma_start(tri_t[:], tri_d)
            for ft in range(2):
                nc.sync.dma_start(wo_t[ft][:], wo_d[ft * 128:(ft + 1) * 128, :])

            # ones/pad columns of v stationaries (during initial DMA wait)
            for st in range(NST):
                v_view = vt[st].rearrange("p (h w) -> p h w", h=HG)
                nc.gpsimd.memset(v_view[:, :, D:D + 1], 1.0)
                nc.gpsimd.memset(v_view[:, :, D + 1:VW], 0.0)

            cos = cs_t[:, 0:S]
            sin = cs_t[:, S:2 * S]

            for sc in range(NSC):
                ssl = slice(sc * SC, (sc + 1) * SC)
                # qk matmuls: pss[jt] = wqk[:,jt-block].T @ x  -> [128, SC]
                pss = [ps1.tile([128, SC], f32, name=f"pss{jt}",
                                tag=f"pss{jt}") for jt in range(4)]
                for e in range(NE):
                    for jt in range(4):
                        nc.tensor.matmul(
                            pss[jt][:],
                            lhsT=wqk_t[:, e * 512 + jt * 128:
                                       e * 512 + (jt + 1) * 128],
                            rhs=xt[:, e * S + sc * SC:e * S + (sc + 1) * SC],
                            start=(e == 0), stop=(e == NE - 1))
                # v matmuls: psv[j] = x[:,st-block].T @ wv -> [128, 256]
                psv = [ps1.tile([128, HG * D], f32, name=f"psv{j}",
                                tag=f"psv{j}") for j in range(4)]
                for e in range(NE):
                    for j in range(4):
                        st = 4 * sc + j
                        nc.tensor.matmul(
                            psv[j][:],
                            lhsT=xt[:, e * S + st * 128:e * S + st * 128 + 128],
                            rhs=wv_t[:, e * 256:(e + 1) * 256],
                            start=(e == 0), stop=(e == NE - 1))
                # evacuate qk psum into qkraw halves (gpsimd cannot
                # read PSUM; scalar is otherwise idle in phase 1)
                nc.vector.tensor_copy(qkraw_q[:, sc * SC:(sc + 1) * SC],
                                      pss[0][:])
                nc.scalar.copy(qkraw_q[:, S + sc * SC:S + (sc + 1) * SC],
                               pss[1][:])
                nc.vector.tensor_copy(qkraw_k[:, sc * SC:(sc + 1) * SC],
                                      pss[2][:])
                nc.scalar.copy(qkraw_k[:, S + sc * SC:S + (sc + 1) * SC],
                               pss[3][:])
                # evacuate v psum into vt stationaries
                for j in range(4):
                    st = 4 * sc + j
                    v_view = vt[st].rearrange("p (h w) -> p h w", h=HG)
                    if j % 2 == 0:
                        nc.vector.tensor_copy(
                            v_view[:, :, 0:D],
                            psv[j].rearrange("p (h d) -> p h d", h=HG))
                    else:
                        nc.scalar.copy(
                            v_view[:, :, 0:D],
                            psv[j].rearrange("p (h d) -> p h d", h=HG))
                # RoPE in place on this chunk's columns
                for t in (qkraw_q, qkraw_k):
                    x0 = t[:, sc * SC:(sc + 1) * SC]
                    x1 = t[:, S + sc * SC:S + (sc + 1) * SC]
                    c_ = cos[:, ssl]
                    s_ = sin[:, ssl]
                    tmp = ptmp.tile([128, SC], f16, tag="rt0")
                    tmp2 = ptmp.tile([128, SC], f16, tag="rt1")
                    nc.vector.tensor_mul(tmp[:], x0, s_)     # x0*sin
                    nc.vector.tensor_mul(tmp2[:], x1, s_)    # x1*sin
                    nc.vector.tensor_mul(x0, x0, c_)         # x0*cos
                    nc.vector.tensor_mul(x1, x1, c_)         # x1*cos
                    nc.vector.tensor_sub(x0, x0, tmp2[:])    # x0 c - x1 s
                    nc.vector.tensor_add(x1, x1, tmp[:])     # x0 s + x1 c
                # repack after each sc-pair: qd/kd[h] rows [x0_h; x1_h]
                if sc % 2 == 1:
                    pc = slice((sc - 1) * SC, (sc + 1) * SC)
                    qs = [nc.sync, nc.gpsimd, nc.sync, nc.gpsimd]
                    for h in range(HG):
                        sl = slice(32 * h, 32 * h + 32)
                        qs[h % 4].dma_start(qd[h][0:32, pc], qkraw_q[sl, pc])
                        qs[(h + 1) % 4].dma_start(
                            qd[h][32:64, pc],
                            qkraw_q[sl, S + (sc - 1) * SC:S + (sc + 1) * SC])
                        qs[(h + 2) % 4].dma_start(kd[h][0:32, pc],
                                                  qkraw_k[sl, pc])
                        qs[(h + 3) % 4].dma_start(
                            kd[h][32:64, pc],
                            qkraw_k[sl, S + (sc - 1) * SC:S + (sc + 1) * SC])

        # ---------------- phase 2: attention + projection ----------------
        # per-pair pipeline (LOOKP decoupling): scores pair -> exp direct
        # from PSUM into fp16 pt (compact, no uninit strips) -> triangle
        # masks on diagonal blocks -> y matmuls 2 pairs behind.
        with ExitStack() as ph2:
            pp = ph2.enter_context(tc.tile_pool(name="pbuf", bufs=4))
            psm = ph2.enter_context(tc.tile_pool(name="small", bufs=3))
            pob = ph2.enter_context(tc.tile_pool(name="outbuf", bufs=4))
            ps_s = ph2.enter_context(
                tc.tile_pool(name="ps_s", bufs=2, space="PSUM"))
            ps_y = ph2.enter_context(
                tc.tile_pool(name="ps_y", bufs=1, space="PSUM"))
            ps_o = ph2.enter_context(
                tc.tile_pool(name="ps_o", bufs=2, space="PSUM"))

            LOOKP = 2
            ob_i = 0

            def pair_blocks(c, pi):
                """[(pt_off, n, rg, t), ...] for the two k-tiles of a pair."""
                out, dst = [], 0
                for half in range(2):
                    t = 2 * pi + half
                    rg = max(0, 128 * (t - 4 * c))
                    n = SC - rg
                    out.append((dst, n, rg, t))
                    dst += n
                return out

            def emit_proj(c):
                nonlocal ob_i
                for st in range(4 * c, 4 * c + 4):
                    for ec in range(2):
                        pso = ps_o.tile([128, SC], f32, tag="pso")
                        for ft in range(2):
                            nc.tensor.matmul(
                                pso[:],
                                lhsT=yT2[ft][:, st * 128:(st + 1) * 128],
                                rhs=wo_t[ft][:, ec * SC:(ec + 1) * SC],
                                start=(ft == 0), stop=(ft == 1))
                        ob = pob.tile([128, SC], f16, tag="ob")
                        nc.vector.tensor_copy(ob[:], pso[:])
                        (nc.sync if ob_i % 2 == 0 else nc.gpsimd).dma_start(
                            out_d[st * 128:(st + 1) * 128,
                                  ec * SC:(ec + 1) * SC],
                            ob[:])
                        ob_i += 1

            for i in range(16):
                c, h = i // HG, i % HG
                npair = 2 * c + 2
                nt = 4 * c + 4
                psy = ps_y.tile([128, SC], f32, tag="psy")
                pts = {}
                for pi in range(npair + LOOKP):
                    if pi < npair:
                        blocks = pair_blocks(c, pi)
                        wtot = blocks[-1][0] + blocks[-1][1]
                        pss2 = ps_s.tile([128, 2 * SC], f32, tag="pss2")
                        pt = pp.tile([128, 2 * SC], f16, tag="pt")
                        for (dst, n, rg, t) in blocks:
                            nc.tensor.matmul(
                                pss2[:, dst:dst + n],
                                lhsT=kd[h][:, t * 128:(t + 1) * 128],
                                rhs=qd[h][:, c * SC + rg:(c + 1) * SC],
                                start=True, stop=True)
                        nc.scalar.activation(pt[:, 0:wtot], pss2[:, 0:wtot],
                                             Exp, bias=ebias_t[:],
                                             scale=ESCALE)
                        for (dst, n, rg, t) in blocks:
                            if t >= 4 * c:  # diagonal block: triangle mask
                                nc.vector.tensor_mul(pt[:, dst:dst + 128],
                                                     pt[:, dst:dst + 128],
                                                     tri_t[:])
                        pts[pi] = (pt, blocks)
                    pp_ = pi - LOOKP
                    if 0 <= pp_ < npair:
                        pt, blocks = pts.pop(pp_)
                        for (dst, n, rg, t) in blocks:
                            nc.tensor.matmul(
                                psy[0:VW, rg:SC],
                                lhsT=vt[t][:, VW * h:VW * (h + 1)],
                                rhs=pt[:, dst:dst + n],
                                start=(t == 0), stop=(t == nt - 1))
                # normalization: denom -> recip -> broadcast -> scale
                ro = 64 * (h % 2)
                nc.vector.tensor_copy(
                    yT[h // 2][ro:ro + 64, c * SC:(c + 1) * SC], psy[0:D, :])
                lrow = psm.tile([1, SC], f32, tag="lrow")
                nc.vector.tensor_copy(lrow[:], psy[D:D + 1, :])
                rrow = psm.tile([1, SC], f32, tag="rrow")
                nc.vector.reciprocal_approx_fast(rrow[:], lrow[:])
                rbc = psm.tile([128, SC], f32, tag="rbc")
                nc.gpsimd.partition_broadcast(rbc[:], rrow[:])
                nc.vector.tensor_mul(
                    yT2[h // 2][ro:ro + 64, c * SC:(c + 1) * SC],
                    yT[h // 2][ro:ro + 64, c * SC:(c + 1) * SC],
                    rbc[ro:ro + 64, :])
                if h == HG - 1:
                    emit_proj(c)

    nc.compile()
    return nc


def _host_inputs(x, W_qkv, W_o):
    """Build the 8 per-core input maps (fp16 device-side compute dtypes)."""
    thetas = 10000.0 ** (-2.0 * (np.arange(D // 2, dtype=np.float32) / D))
    freqs = np.arange(S, dtype=np.float32)[:, None] * thetas[None, :]  # [S, 32]
    cosT = np.cos(freqs).astype(np.float32).T  # [32, S]
    sinT = np.sin(freqs).astype(np.float32).T
    cs = np.ascontiguousarray(np.concatenate(
        [np.tile(cosT, (4, 1)), np.tile(sinT, (4, 1))], axis=1)
        .astype(np.float16))  # [128, 2S]

    jj = np.arange(128)[:, None]
    tri = np.ascontiguousarray(
        (jj <= np.arange(128)[None, :]).astype(np.float16))  # [128, 128]

    # [4*E, SC]: chunk sc contiguous, rows (p, e)-ordered to match the
    # SBUF xt view [128 p, 8 e, 512]
    xTs = [np.ascontiguousarray(
        x[b].T.astype(np.float16).reshape(NE, 128, NSC, SC)
        .transpose(2, 1, 0, 3).reshape(NSC * E, SC)) for b in range(B)]

    in_maps = []
    for core in range(NCORES):
        b, hg = core // 4, core % 4
        heads = range(hg * HG, (hg + 1) * HG)
        qx0 = [h * D + 2 * m for h in heads for m in range(D // 2)]
        qx1 = [h * D + 2 * m + 1 for h in heads for m in range(D // 2)]
        rows = (qx0 + qx1 + [E + i for i in qx0] + [E + i for i in qx1])
        wqk = np.ascontiguousarray(W_qkv[rows].T.astype(np.float16))  # [E, 512]
        vrows = [2 * E + h * D + d for h in heads for d in range(D)]
        wv = np.ascontiguousarray(W_qkv[vrows].T.astype(np.float16))  # [E, 256]
        wo = np.ascontiguousarray(
            W_o[:, hg * HG * D:(hg + 1) * HG * D].T.astype(np.float16))
        in_maps.append({
            "xT": xTs[b], "wqk": wqk, "wv": wv, "wo": wo,
            "cs": cs, "tri": tri,
        })
    return in_maps


def kernel(x, W_qkv, W_o):
    global _COMPILED
    x = np.ascontiguousarray(np.asarray(x, dtype=np.float32))
    W_qkv = np.ascontiguousarray(np.asarray(W_qkv, dtype=np.float32))
    W_o = np.ascontiguousarray(np.asarray(W_o, dtype=np.float32))

    if _COMPILED is None:
        _COMPILED = _build_bass()
    nc = _COMPILED

    from concourse.bass_utils import run_bass_kernel_spmd
    in_maps = _host_inputs(x, W_qkv, W_o)
    res = run_bass_kernel_spmd(nc, in_maps, core_ids=list(range(NCORES)))
    out = np.zeros((B, S, E), dtype=np.float32)
    for core in range(NCORES):
        out[core // 4] += res.results[core]["out"].astype(np.float32)
    return out
